# revision 1
# baseline (speedup 1.0000x reference)
"""Compressed Interaction Network (CIN) forward on 8 Trainium2 NeuronCores.

Math (per batch item, m=32 fields, d=64 embed, H=256 hidden):
    x0 = x[i]                          # (m, d)
    h  = x0
    layer l in 0..2:
        z = outer(x0, h) over d        # (m*n, d), z[(a,b),:] = x0[a,:]*h[b,:]
        y = relu(W_l^T z + b_l)        # (H, d)
        xcur, h = split_half(y) (layers 0,1); xcur = h = y (layer 2)
    f = concat(xcur_0, xcur_1, xcur_2) # (512, d)
    out[i] = sum_d(f) @ fc_W + fc_b    # scalar

Mapping: batch 1024 -> 8 cores x 128 items, 16 groups of 8 items per core.
 - Outer-product operands are built with DMA broadcast reads (stride-0 APs),
   one DMA per tile so consumers wait on a single DMA semaphore lane.
 - z tiles computed on VectorE in fp16 (2x mode), layout [k-part, (i, m, d)].
 - Conv matmuls on PE: stationary W chunks [128, 128] fp16, moving z
   [128, 512] (8 items x 64 d), accumulated over k-chunks in fp32 PSUM.
 - Bias+ReLU fused into the PSUM->SBUF move on ScalarE; per-item d-sums for
   the final FC are fused there too via accum_out.
 - Final dot: PE matmul of [128,1] fc weight chunks against [128, 128] sums.
"""

import numpy as np

import concourse.bass as bass
import concourse.tile as tile
from concourse import mybir
from concourse.bass_utils import run_bass_kernel_spmd

N_CORES = 8
B_TOTAL = 1024
B_CORE = B_TOTAL // N_CORES  # 128
M = 32  # num fields
D = 64  # embed dim
H = 256  # conv output channels
GROUP = 8  # items per group (512 moving columns)
N_GROUPS = B_CORE // GROUP  # 16
MD = M * D  # 2048, elements per item row

F16 = mybir.dt.float16
F32 = mybir.dt.float32
RELU = mybir.ActivationFunctionType.Relu
IDENT = mybir.ActivationFunctionType.Identity


def build():
    nc = bass.Bass()
    xh = nc.declare_dram_parameter("xh", [B_CORE, M, D], F16, isOutput=False)
    # x rows tiled 4x along the field axis: xr[i, p, d] = x[i, p % 32, d]
    xr = nc.declare_dram_parameter("xr", [B_CORE, 128, D], F16, isOutput=False)
    w0 = nc.declare_dram_parameter("w0", [8, 128, H], F16, isOutput=False)
    w1 = nc.declare_dram_parameter("w1", [32, 128, H], F16, isOutput=False)
    w2 = nc.declare_dram_parameter("w2", [32, 128, H], F16, isOutput=False)
    bia = nc.declare_dram_parameter("bia", [128, 3, 2], F32, isOutput=False)
    fcw = nc.declare_dram_parameter("fcw", [128, 4], F32, isOutput=False)
    fcb = nc.declare_dram_parameter("fcb", [1, 1], F32, isOutput=False)
    out = nc.declare_dram_parameter("out", [B_CORE, 1], F32, isOutput=True)

    with tile.TileContext(nc) as tc:
        with (
            tc.tile_pool(name="consts", bufs=1) as consts,
            tc.tile_pool(name="bpool", bufs=2) as bpool,
            tc.tile_pool(name="epool", bufs=4) as epool,
            tc.tile_pool(name="zpool", bufs=3) as zpool,
            tc.tile_pool(name="hpool", bufs=4) as hpool,
            tc.tile_pool(name="spool", bufs=1) as spool,
            tc.tile_pool(name="ppool", bufs=6, space="PSUM") as ppool,
            tc.tile_pool(name="fcp", bufs=1, space="PSUM") as fcp,
        ):
            # --- resident constants ---
            w0_sb = consts.tile([128, 8, H], F16, tag="w0")
            nc.sync.dma_start(w0_sb[:], w0[:].rearrange("c k o -> k c o"))
            w1_sb = consts.tile([128, 32, H], F16, tag="w1")
            nc.sync.dma_start(w1_sb[:], w1[:].rearrange("c k o -> k c o"))
            w2_sb = consts.tile([128, 32, H], F16, tag="w2")
            nc.sync.dma_start(w2_sb[:], w2[:].rearrange("c k o -> k c o"))
            bia_sb = consts.tile([128, 3, 2], F32, tag="bia")
            nc.sync.dma_start(bia_sb[:], bia[:])
            fcw_sb = consts.tile([128, 4], F32, tag="fcw")
            nc.sync.dma_start(fcw_sb[:], fcw[:])
            fcb_sb = consts.tile([1, 1], F32, tag="fcb")
            nc.sync.dma_start(fcb_sb[:], fcb[:])

            # per-item d-sums of the relu'd xs channels, [channel, item]
            s_tiles = [
                spool.tile([128, B_CORE], F32, tag=f"s{c}", name=f"s{c}")
                for c in range(4)
            ]

            for g in range(N_GROUPS):
                i0 = g * GROUP

                # B[p, i, m, d] = x_i[m, d] for every partition p
                # (one DMA: partition loop stride 0, (m d) merged contiguous)
                Bg = bpool.tile([128, GROUP, M, D], F16, tag="B")
                src = bass.AP(
                    tensor=xh,
                    offset=i0 * MD,
                    ap=[[0, 128], [MD, GROUP], [1, MD]],
                )
                nc.sync.dma_start(Bg[:], src)

                # R[p, i, d] = x_i[p % 32, d]  (from the host-tiled copy)
                Rg = epool.tile([128, GROUP, D], F16, tag="R")
                src = bass.AP(
                    tensor=xr,
                    offset=i0 * 128 * D,
                    ap=[[D, 128], [128 * D, GROUP], [1, D]],
                )
                nc.sync.dma_start(Rg[:], src)

                # ---------- layer 0: z0[(a,b)] = x[4c + p//32] * x[p%32] ----------
                # per 32-partition block s: z0[32s:32s+32] = R-block * B[:, :, 4c+s]
                # (operands of an engine op must share the partition range)
                ps0 = [
                    ppool.tile([128, GROUP * D], F32, tag="yps", name="ps0")
                    for _ in range(2)
                ]
                for c in range(8):
                    z0 = epool.tile([128, GROUP, D], F16, tag="z0")
                    for s in range(4):
                        pr = slice(32 * s, 32 * (s + 1))
                        nc.vector.tensor_mul(
                            z0[pr, :, :],
                            Rg[pr, :, :],
                            Bg[pr, :, 4 * c + s, :],
                        )
                    for oc in range(2):
                        nc.tensor.matmul(
                            ps0[oc][:],
                            w0_sb[:, c, oc * 128 : (oc + 1) * 128],
                            z0[:],
                            start=(c == 0),
                            stop=(c == 7),
                        )

                # psum -> sbuf with fused bias+relu; chunk1 becomes next h,
                # chunk0 only needs its per-item d-sums (accum_out into s0)
                h1 = hpool.tile([128, GROUP, D], F16, tag="h1")
                nc.scalar.activation(h1[:], ps0[1][:], RELU, bias=bia_sb[:, 0, 1:2])
                for i in range(GROUP):
                    sl = slice(i * D, (i + 1) * D)
                    nc.scalar.activation(
                        ps0[0][:, sl],
                        ps0[0][:, sl],
                        RELU,
                        bias=bia_sb[:, 0, 0:1],
                        accum_out=s_tiles[0][:, i0 + i : i0 + i + 1],
                    )

                # ---------- layers 1 and 2 ----------
                for lay in range(2):
                    w_sb = w1_sb if lay == 0 else w2_sb
                    h_in = h1 if lay == 0 else h2
                    ps = [
                        ppool.tile([128, GROUP * D], F32, tag="yps", name="ps")
                        for _ in range(2)
                    ]
                    for mb in range(8):
                        zt = zpool.tile([128, GROUP, 4, D], F16, tag="z")
                        nc.vector.tensor_mul(
                            zt[:],
                            h_in[:, :, None, :].to_broadcast((128, GROUP, 4, D)),
                            Bg[:, :, 4 * mb : 4 * mb + 4, :],
                        )
                        for mm in range(4):
                            m = 4 * mb + mm
                            for oc in range(2):
                                nc.tensor.matmul(
                                    ps[oc][:],
                                    w_sb[:, m, oc * 128 : (oc + 1) * 128],
                                    zt[:, :, mm, :],
                                    start=(m == 0),
                                    stop=(m == 31),
                                )
                    if lay == 0:
                        # split_half: chunk0 -> s1 sums, chunk1 -> h2
                        h2 = hpool.tile([128, GROUP, D], F16, tag="h2")
                        nc.scalar.activation(
                            h2[:], ps[1][:], RELU, bias=bia_sb[:, 1, 1:2]
                        )
                        for i in range(GROUP):
                            sl = slice(i * D, (i + 1) * D)
                            nc.scalar.activation(
                                ps[0][:, sl],
                                ps[0][:, sl],
                                RELU,
                                bias=bia_sb[:, 1, 0:1],
                                accum_out=s_tiles[1][:, i0 + i : i0 + i + 1],
                            )
                    else:
                        # last layer: both chunks feed the FC sums (s2, s3)
                        for oc in range(2):
                            for i in range(GROUP):
                                sl = slice(i * D, (i + 1) * D)
                                nc.scalar.activation(
                                    ps[oc][:, sl],
                                    ps[oc][:, sl],
                                    RELU,
                                    bias=bia_sb[:, 2, oc : oc + 1],
                                    accum_out=s_tiles[2 + oc][
                                        :, i0 + i : i0 + i + 1
                                    ],
                                )

            # ---------- final FC: out[i] = sum_c fcw[c] * s[c, i] + fcb ----------
            fc_ps = fcp.tile([1, B_CORE], F32, tag="fc")
            for c in range(4):
                nc.tensor.matmul(
                    fc_ps[:],
                    fcw_sb[:, c : c + 1],
                    s_tiles[c][:],
                    start=(c == 0),
                    stop=(c == 3),
                )
            osb = consts.tile([1, B_CORE], F32, tag="osb")
            nc.scalar.activation(osb[:], fc_ps[:], IDENT, bias=fcb_sb[0:1, 0:1])
            nc.sync.dma_start(out[:], osb[:])

    _legalize_waits(nc)
    return nc


def _legalize_waits(nc, max_waits=1):
    """walrus codegen allows at most 2 semaphore waits per instruction; spill
    the excess onto NoOps injected just before the offender on the same
    engine (same-engine FIFO makes this ordering-equivalent)."""
    for bb in nc.main_func.blocks:
        insts = bb.instructions
        i = 0
        new_list = []
        changed = False
        for ins in insts:
            si = ins.sync_info
            if si is not None and si.on_wait and len(si.on_wait) > max_waits:
                waits = list(si.on_wait)
                extra, keep = waits[:-max_waits], waits[-max_waits:]
                k = 0
                while k < len(extra):
                    chunk = extra[k : k + max_waits]
                    nop = mybir.InstNoOp(name=f"{ins.name}-w{k}", ins=[], outs=[])
                    nop.engine = ins.engine
                    nop.sync_info = mybir.SyncInfo(on_wait=chunk, on_update=[])
                    new_list.append(nop)
                    k += max_waits
                ins.sync_info = mybir.SyncInfo(
                    on_wait=keep,
                    on_update=list(si.on_update) if si.on_update else [],
                )
                changed = True
            new_list.append(ins)
        if changed:
            if hasattr(bb, "set_instructions"):
                bb.set_instructions(new_list)
            else:
                insts.clear()
                insts.extend(new_list)
                if len(bb.instructions) != len(new_list):
                    bb.instructions = new_list


def prep_inputs(x, W0, b0, W1, b1, W2, b2, fc_W, fc_b):
    """Host-side reshape/cast into the per-core input maps."""
    xh = np.ascontiguousarray(x.astype(np.float16))
    xr = np.ascontiguousarray(
        np.tile(xh.reshape(B_TOTAL, 1, M, D), (1, 4, 1, 1)).reshape(
            B_TOTAL, 128, D
        )
    )
    w0 = np.ascontiguousarray(W0.astype(np.float16).reshape(8, 128, H))
    w1 = np.ascontiguousarray(W1.astype(np.float16).reshape(32, 128, H))
    w2 = np.ascontiguousarray(W2.astype(np.float16).reshape(32, 128, H))
    bia = np.ascontiguousarray(
        np.stack([b0, b1, b2]).reshape(3, 2, 128).transpose(2, 0, 1).astype(np.float32)
    )
    fcw = np.ascontiguousarray(fc_W.reshape(4, 128).T.astype(np.float32))
    fcb = np.ascontiguousarray(fc_b.reshape(1, 1).astype(np.float32))
    shared = {"w0": w0, "w1": w1, "w2": w2, "bia": bia, "fcw": fcw, "fcb": fcb}
    return [
        {
            "xh": xh[i * B_CORE : (i + 1) * B_CORE],
            "xr": xr[i * B_CORE : (i + 1) * B_CORE],
            **shared,
        }
        for i in range(N_CORES)
    ]


_NC = None


def _get_nc():
    global _NC
    if _NC is None:
        _NC = build()
    return _NC


def kernel(**inputs):
    in_maps = prep_inputs(**inputs)
    res = run_bass_kernel_spmd(_get_nc(), in_maps, list(range(N_CORES)))
    return np.ascontiguousarray(
        np.concatenate([r["out"] for r in res.results], axis=0).astype(np.float32)
    )



# revision 6
# speedup vs baseline: 1.4029x; 1.4029x over previous
"""Compressed Interaction Network (CIN) forward on 8 Trainium2 NeuronCores.

Math (per batch item, m=32 fields, d=64 embed, H=256 hidden):
    x0 = x[i]                          # (m, d)
    h  = x0
    layer l in 0..2:
        z = outer(x0, h) over d        # (m*n, d), z[(a,b),:] = x0[a,:]*h[b,:]
        y = relu(W_l^T z + b_l)        # (H, d)
        xcur, h = split_half(y) (layers 0,1); xcur = h = y (layer 2)
    f = concat(xcur_0, xcur_1, xcur_2) # (512, d)
    out[i] = sum_d(f) @ fc_W + fc_b    # scalar

Mapping: batch 1024 -> 8 cores x 128 items, 16 groups of 8 items per core.

v2 design (vs v1):
 - Layer 0 exploits z symmetry: z[(a,b)] == z[(b,a)], so W0 is folded to
   528 unique pairs (padded to 640 = 5 k-chunks).  Host prepares xpa/xpb
   gather layouts so z0 is ONE full-width DVE op per group (v1 used 32
   quarter-width ops).
 - Per-item d-sums for the final FC are DVE X-axis reduces over the
   relu'd SBUF copy (v1: 32 tiny ScalarE accum ops per group, ~455ns ea).
 - Explicit 3-stage pipeline: iteration t runs layer0(g=t), layer1(g=t-1),
   layer2(g=t-2) so the h->z dependency chain of one group overlaps the
   matmuls of other groups; PE stays dense and HAM-warm.
 - z for layers 1/2 built in 4 sub-tiles of 8 k-chunks each so PE can
   start consuming while DVE still builds the rest.
"""

import numpy as np

import concourse.bass as bass
import concourse.tile as tile
from concourse import mybir
from concourse.bass_utils import run_bass_kernel_spmd

N_CORES = 8
B_TOTAL = 1024
B_CORE = B_TOTAL // N_CORES  # 128
M = 32  # num fields
D = 64  # embed dim
H = 256  # conv output channels
GROUP = 8  # items per group (512 moving columns)
N_GROUPS = B_CORE // GROUP  # 16
MD = M * D  # 2048, elements per item row
PAIR_CHUNKS = 5  # 528 symmetric pairs padded to 640 = 5 * 128

F16 = mybir.dt.float16
F32 = mybir.dt.float32
RELU = mybir.ActivationFunctionType.Relu
IDENT = mybir.ActivationFunctionType.Identity
AXX = mybir.AxisListType.X


def build():
    nc = bass.Bass()
    # xg[g, m, i, d] = x[8g + i, m, d] -- per-group contiguous broadcast source
    xg = nc.declare_dram_parameter("xg", [N_GROUPS, M * GROUP * D], F16, isOutput=False)
    # symmetric-pair gather layouts: xpa[g, p, (c, i, d)] = x[8g+i, ia[128c+p], d]
    xpa = nc.declare_dram_parameter(
        "xpa", [N_GROUPS, 128, PAIR_CHUNKS * GROUP * D], F16, isOutput=False
    )
    xpb = nc.declare_dram_parameter(
        "xpb", [N_GROUPS, 128, PAIR_CHUNKS * GROUP * D], F16, isOutput=False
    )
    w0 = nc.declare_dram_parameter("w0", [PAIR_CHUNKS, 128, H], F16, isOutput=False)
    w1 = nc.declare_dram_parameter("w1", [32, 128, H], F16, isOutput=False)
    w2 = nc.declare_dram_parameter("w2", [32, 128, H], F16, isOutput=False)
    bia = nc.declare_dram_parameter("bia", [128, 3, 2], F32, isOutput=False)
    fcw = nc.declare_dram_parameter("fcw", [128, 4], F32, isOutput=False)
    fcb = nc.declare_dram_parameter("fcb", [1, 1], F32, isOutput=False)
    out = nc.declare_dram_parameter("out", [B_CORE, 1], F32, isOutput=True)

    with tile.TileContext(nc) as tc:
        with (
            tc.tile_pool(name="consts", bufs=1) as consts,
            tc.tile_pool(name="bpool", bufs=3) as bpool,
            tc.tile_pool(name="xppool", bufs=2) as xppool,
            tc.tile_pool(name="z0pool", bufs=2) as z0pool,
            tc.tile_pool(name="ztpool", bufs=4) as ztpool,
            tc.tile_pool(name="hpool", bufs=2) as hpool,
            tc.tile_pool(name="ypool", bufs=4) as ypool,
            tc.tile_pool(name="spool", bufs=1) as spool,
            tc.tile_pool(name="ppool", bufs=6, space="PSUM") as ppool,
            tc.tile_pool(name="fcp", bufs=1, space="PSUM") as fcp,
        ):
            # --- resident constants ---
            w0_sb = consts.tile([128, PAIR_CHUNKS, H], F16, tag="w0")
            nc.sync.dma_start(w0_sb[:], w0[:].rearrange("c k o -> k c o"))
            w1_sb = consts.tile([128, 32, H], F16, tag="w1")
            nc.sync.dma_start(w1_sb[:], w1[:].rearrange("c k o -> k c o"))
            w2_sb = consts.tile([128, 32, H], F16, tag="w2")
            nc.sync.dma_start(w2_sb[:], w2[:].rearrange("c k o -> k c o"))
            bia_sb = consts.tile([128, 3, 2], F32, tag="bia")
            nc.sync.dma_start(bia_sb[:], bia[:])
            fcw_sb = consts.tile([128, 4], F32, tag="fcw")
            nc.sync.dma_start(fcw_sb[:], fcw[:])
            fcb_sb = consts.tile([1, 1], F32, tag="fcb")
            nc.sync.dma_start(fcb_sb[:], fcb[:])

            # per-item d-sums of the relu'd xs channels, [channel, item]
            s_tiles = [
                spool.tile([128, B_CORE], F32, tag=f"s{c}", name=f"s{c}")
                for c in range(4)
            ]

            h1s = [None] * N_GROUPS
            h2s = [None] * N_GROUPS
            Bgs = [None] * N_GROUPS

            def stage_a(g):
                """layer 0 for group g: DMAs, z0 build, 10 matmuls, h1/y0."""
                i0 = g * GROUP
                # B[p, m, i, d] = x_{8g+i}[m, d] for every partition p (one
                # DMA, partition stride 0).  m-major so layer-1/2 slices are
                # by m.
                Bg = bpool.tile([128, M, GROUP, D], F16, tag="B")
                nc.sync.dma_start(
                    Bg[:],
                    bass.AP(
                        tensor=xg,
                        offset=g * M * GROUP * D,
                        ap=[[0, 128], [1, M * GROUP * D]],
                    ),
                )
                Bgs[g] = Bg

                npair = PAIR_CHUNKS * GROUP * D
                xa = xppool.tile([128, PAIR_CHUNKS, GROUP, D], F16, tag="xa")
                nc.sync.dma_start(
                    xa[:],
                    bass.AP(
                        tensor=xpa,
                        offset=g * 128 * npair,
                        ap=[[npair, 128], [1, npair]],
                    ),
                )
                xb = xppool.tile([128, PAIR_CHUNKS, GROUP, D], F16, tag="xb")
                nc.sync.dma_start(
                    xb[:],
                    bass.AP(
                        tensor=xpb,
                        offset=g * 128 * npair,
                        ap=[[npair, 128], [1, npair]],
                    ),
                )

                # z0[p, c, i, d] = xa * xb  -- one full-width DVE op
                z0 = z0pool.tile([128, PAIR_CHUNKS, GROUP, D], F16, tag="z0")
                nc.vector.tensor_mul(z0[:], xa[:], xb[:])

                ps = [
                    ppool.tile([128, GROUP * D], F32, tag="yps", name=f"ps0_{g}_{oc}")
                    for oc in range(2)
                ]
                for c in range(PAIR_CHUNKS):
                    for oc in range(2):
                        nc.tensor.matmul(
                            ps[oc][:],
                            w0_sb[:, c, oc * 128 : (oc + 1) * 128],
                            z0[:, c, :, :],
                            start=(c == 0),
                            stop=(c == PAIR_CHUNKS - 1),
                        )

                # h-half -> h1 (feeds layer-1 z); x-half -> y0 + d-sums
                h1 = hpool.tile([128, GROUP, D], F16, tag="h1")
                nc.scalar.activation(h1[:], ps[1][:], RELU, bias=bia_sb[:, 0, 1:2])
                h1s[g] = h1
                y0 = ypool.tile([128, GROUP, D], F16, tag="y")
                nc.scalar.activation(y0[:], ps[0][:], RELU, bias=bia_sb[:, 0, 0:1])
                nc.vector.reduce_sum(s_tiles[0][:, i0 : i0 + GROUP], y0[:], axis=AXX)

            def stage_bc(g, lay):
                """layer 1 (lay=0) / layer 2 (lay=1) for group g."""
                i0 = g * GROUP
                w_sb = w1_sb if lay == 0 else w2_sb
                h_in = h1s[g] if lay == 0 else h2s[g]
                Bg = Bgs[g]
                ps = [
                    ppool.tile(
                        [128, GROUP * D], F32, tag="yps", name=f"ps{lay + 1}_{g}_{oc}"
                    )
                    for oc in range(2)
                ]
                for j in range(4):  # 4 sub-tiles of 8 k-chunks
                    zt = ztpool.tile([128, 8, GROUP, D], F16, tag="zt")
                    nc.vector.tensor_mul(
                        zt[:],
                        h_in[:, None, :, :].to_broadcast((128, 8, GROUP, D)),
                        Bg[:, 8 * j : 8 * j + 8, :, :],
                    )
                    for mm in range(8):
                        m = 8 * j + mm
                        for oc in range(2):
                            nc.tensor.matmul(
                                ps[oc][:],
                                w_sb[:, m, oc * 128 : (oc + 1) * 128],
                                zt[:, mm, :, :],
                                start=(m == 0),
                                stop=(m == 31),
                            )
                if lay == 0:
                    # split_half: x-half -> s1 sums, h-half -> h2
                    h2 = hpool.tile([128, GROUP, D], F16, tag="h2")
                    nc.scalar.activation(h2[:], ps[1][:], RELU, bias=bia_sb[:, 1, 1:2])
                    h2s[g] = h2
                    y1 = ypool.tile([128, GROUP, D], F16, tag="y")
                    nc.scalar.activation(y1[:], ps[0][:], RELU, bias=bia_sb[:, 1, 0:1])
                    nc.vector.reduce_sum(
                        s_tiles[1][:, i0 : i0 + GROUP], y1[:], axis=AXX
                    )
                else:
                    # last layer: both halves feed the FC sums (s2, s3)
                    for oc in range(2):
                        y2 = ypool.tile([128, GROUP, D], F16, tag="y")
                        nc.scalar.activation(
                            y2[:], ps[oc][:], RELU, bias=bia_sb[:, 2, oc : oc + 1]
                        )
                        nc.vector.reduce_sum(
                            s_tiles[2 + oc][:, i0 : i0 + GROUP], y2[:], axis=AXX
                        )

            # 3-stage pipeline: A[t] | B[t-1] | C[t-2]
            for t in range(N_GROUPS + 2):
                if t < N_GROUPS:
                    stage_a(t)
                if 1 <= t <= N_GROUPS:
                    stage_bc(t - 1, 0)
                if t >= 2:
                    stage_bc(t - 2, 1)

            # ---------- final FC: out[i] = sum_c fcw[c] * s[c, i] + fcb ----------
            fc_ps = fcp.tile([1, B_CORE], F32, tag="fc")
            for c in range(4):
                nc.tensor.matmul(
                    fc_ps[:],
                    fcw_sb[:, c : c + 1],
                    s_tiles[c][:],
                    start=(c == 0),
                    stop=(c == 3),
                )
            osb = consts.tile([1, B_CORE], F32, tag="osb")
            nc.scalar.activation(osb[:], fc_ps[:], IDENT, bias=fcb_sb[0:1, 0:1])
            nc.sync.dma_start(out[:], osb[:])

    _legalize_waits(nc)
    return nc


def _legalize_waits(nc, max_waits=1):
    """walrus codegen allows at most 2 semaphore waits per instruction; spill
    the excess onto NoOps injected just before the offender on the same
    engine (same-engine FIFO makes this ordering-equivalent)."""
    for bb in nc.main_func.blocks:
        insts = bb.instructions
        new_list = []
        changed = False
        for ins in insts:
            si = ins.sync_info
            if si is not None and si.on_wait and len(si.on_wait) > max_waits:
                waits = list(si.on_wait)
                extra, keep = waits[:-max_waits], waits[-max_waits:]
                k = 0
                while k < len(extra):
                    chunk = extra[k : k + max_waits]
                    nop = mybir.InstNoOp(name=f"{ins.name}-w{k}", ins=[], outs=[])
                    nop.engine = ins.engine
                    nop.sync_info = mybir.SyncInfo(on_wait=chunk, on_update=[])
                    new_list.append(nop)
                    k += max_waits
                ins.sync_info = mybir.SyncInfo(
                    on_wait=keep,
                    on_update=list(si.on_update) if si.on_update else [],
                )
                changed = True
            new_list.append(ins)
        if changed:
            if hasattr(bb, "set_instructions"):
                bb.set_instructions(new_list)
            else:
                insts.clear()
                insts.extend(new_list)
                if len(bb.instructions) != len(new_list):
                    bb.instructions = new_list


def _sym_pairs():
    """Enumeration of the 528 unique (a<=b) field pairs, zero-padded to 640."""
    ia, ib = [], []
    for a in range(M):
        for b in range(a, M):
            ia.append(a)
            ib.append(b)
    pad = PAIR_CHUNKS * 128 - len(ia)
    ia += [0] * pad
    ib += [0] * pad
    return np.asarray(ia), np.asarray(ib), pad


def prep_inputs(x, W0, b0, W1, b1, W2, b2, fc_W, fc_b):
    """Host-side reshape/cast into the per-core input maps."""
    xh = np.ascontiguousarray(np.asarray(x).astype(np.float16))
    ia, ib, pad = _sym_pairs()
    ng_all = B_TOTAL // GROUP
    # xg[g, m, i, d] = x[8g + i, m, d]
    xg = np.ascontiguousarray(
        xh.reshape(ng_all, GROUP, M, D).transpose(0, 2, 1, 3)
    ).reshape(ng_all, M * GROUP * D)
    # xpa[g, p, c, i, d] = x[8g + i, ia[128c + p], d]
    xpa = np.ascontiguousarray(
        xh[:, ia, :]
        .reshape(ng_all, GROUP, PAIR_CHUNKS, 128, D)
        .transpose(0, 3, 2, 1, 4)
    ).reshape(ng_all, 128, PAIR_CHUNKS * GROUP * D)
    xpb = np.ascontiguousarray(
        xh[:, ib, :]
        .reshape(ng_all, GROUP, PAIR_CHUNKS, 128, D)
        .transpose(0, 3, 2, 1, 4)
    ).reshape(ng_all, 128, PAIR_CHUNKS * GROUP * D)
    # fold W0 over the (a,b)<->(b,a) symmetry: row (a,b) gets W0[a*32+b]
    # (+ W0[b*32+a] when a != b); padded rows are zero.
    W0 = np.asarray(W0, dtype=np.float64)
    w0f = W0[ia * M + ib] + np.where((ia != ib)[:, None], W0[ib * M + ia], 0.0)
    w0f[len(ia) - pad :] = 0.0
    w0f = np.ascontiguousarray(
        w0f.astype(np.float16).reshape(PAIR_CHUNKS, 128, H)
    )
    w1 = np.ascontiguousarray(np.asarray(W1).astype(np.float16).reshape(32, 128, H))
    w2 = np.ascontiguousarray(np.asarray(W2).astype(np.float16).reshape(32, 128, H))
    bia = np.ascontiguousarray(
        np.stack([b0, b1, b2]).reshape(3, 2, 128).transpose(2, 0, 1).astype(np.float32)
    )
    fcw = np.ascontiguousarray(np.asarray(fc_W).reshape(4, 128).T.astype(np.float32))
    fcb = np.ascontiguousarray(np.asarray(fc_b).reshape(1, 1).astype(np.float32))
    shared = {"w0": w0f, "w1": w1, "w2": w2, "bia": bia, "fcw": fcw, "fcb": fcb}
    return [
        {
            "xg": xg[i * N_GROUPS : (i + 1) * N_GROUPS],
            "xpa": xpa[i * N_GROUPS : (i + 1) * N_GROUPS],
            "xpb": xpb[i * N_GROUPS : (i + 1) * N_GROUPS],
            **shared,
        }
        for i in range(N_CORES)
    ]


_NC = None


def _get_nc():
    global _NC
    if _NC is None:
        _NC = build()
    return _NC


def kernel(**inputs):
    in_maps = prep_inputs(**inputs)
    res = run_bass_kernel_spmd(_get_nc(), in_maps, list(range(N_CORES)))
    return np.ascontiguousarray(
        np.concatenate([r["out"] for r in res.results], axis=0).astype(np.float32)
    )


# revision 12
# speedup vs baseline: 1.6799x; 1.1974x over previous
"""Compressed Interaction Network (CIN) forward on 8 Trainium2 NeuronCores.

Math (per batch item, m=32 fields, d=64 embed, H=256 hidden):
    x0 = x[i]                          # (m, d)
    h  = x0
    layer l in 0..2:
        z = outer(x0, h) over d        # (m*n, d), z[(a,b),:] = x0[a,:]*h[b,:]
        y = relu(W_l^T z + b_l)        # (H, d)
        xcur, h = split_half(y) (layers 0,1); xcur = h = y (layer 2)
    f = concat(xcur_0, xcur_1, xcur_2) # (512, d)
    out[i] = sum_d(f) @ fc_W + fc_b    # scalar

Mapping: batch 1024 -> 8 cores x 128 items, 16 groups of 8 items per core.

Design notes:
 - Layer 0 exploits z symmetry: z[(a,b)] == z[(b,a)], so W0 is folded to
   528 unique pairs (padded to 640 = 5 k-chunks); host prepares xpa/xpb
   gather layouts so z0 is ONE full-width DVE op per group.
 - Per-item d-sums for the final FC are DVE X-axis reduces over the
   relu'd SBUF copy.
 - Explicit 3-stage pipeline: iteration t runs layer0(g=t), layer1(g=t-1),
   layer2(g=t-2) so the h->z dependency chain of one group overlaps the
   matmuls of other groups; PE stays dense and HAM-warm.
 - Bg / w1 / w2 split into 8-m chunks with separate DMAs so consumers
   start as soon as their chunk lands (fast pipeline fill).
 - NDR (0..4): number of 8-m sub-tiles per layer-1/2 group converted to
   fp8e4 on ScalarE and consumed as DoubleRow matmuls (2 k-chunks per
   PE pass).  fp8 weights are pre-scaled by WS=256 to stay out of the
   e4m3 subnormal range; the psum is descaled in the activation.
"""

import numpy as np
import ml_dtypes

import concourse.bass as bass
import concourse.tile as tile
from concourse import mybir
from concourse.bass_utils import run_bass_kernel_spmd

N_CORES = 8
B_TOTAL = 1024
B_CORE = B_TOTAL // N_CORES  # 128
M = 32  # num fields
D = 64  # embed dim
H = 256  # conv output channels
GROUP = 8  # items per group (512 moving columns)
N_GROUPS = B_CORE // GROUP  # 16
MD = M * D  # 2048, elements per item row
PAIR_CHUNKS = 5  # 528 symmetric pairs padded to 640 = 5 * 128

NDR = 2  # 8-m sub-tiles per layer-1/2 group run as fp8 DoubleRow (0..4)
WS = 256.0 if NDR else 1.0  # layer-1/2 weight pre-scale (fp8 subnormal dodge)

F16 = mybir.dt.float16
F32 = mybir.dt.float32
F8 = mybir.dt.float8e4
RELU = mybir.ActivationFunctionType.Relu
IDENT = mybir.ActivationFunctionType.Identity
AXX = mybir.AxisListType.X
DR = mybir.MatmulPerfMode.DoubleRow


def build():
    nc = bass.Bass()
    # xg[g, m, i, d] = x[8g + i, m, d] -- per-group contiguous broadcast source
    xg = nc.declare_dram_parameter("xg", [N_GROUPS, M * GROUP * D], F16, isOutput=False)
    # symmetric-pair gather layouts: xpa[g, p, (c, i, d)] = x[8g+i, ia[128c+p], d]
    xpa = nc.declare_dram_parameter(
        "xpa", [N_GROUPS, 128, PAIR_CHUNKS * GROUP * D], F16, isOutput=False
    )
    xpb = nc.declare_dram_parameter(
        "xpb", [N_GROUPS, 128, PAIR_CHUNKS * GROUP * D], F16, isOutput=False
    )
    w0 = nc.declare_dram_parameter("w0", [PAIR_CHUNKS, 128, H], F16, isOutput=False)
    w1 = nc.declare_dram_parameter("w1", [32, 128, H], F16, isOutput=False)
    w2 = nc.declare_dram_parameter("w2", [32, 128, H], F16, isOutput=False)
    if NDR:
        # oc-major fp8 weights: row oc*(NDR*8) + c holds k-chunk c of output
        # half oc -- each DR weight tile is then a clean [128, 8, 128] (an
        # oc-sliced wider tile crashes the PE in DoubleRow mode).
        w1q = nc.declare_dram_parameter(
            "w1q", [2 * NDR * 8, 128, H // 2], F8, isOutput=False
        )
        w2q = nc.declare_dram_parameter(
            "w2q", [2 * NDR * 8, 128, H // 2], F8, isOutput=False
        )
    bia = nc.declare_dram_parameter("bia", [128, 3, 2], F32, isOutput=False)
    fcw = nc.declare_dram_parameter("fcw", [128, 4], F32, isOutput=False)
    fcb = nc.declare_dram_parameter("fcb", [1, 1], F32, isOutput=False)
    out = nc.declare_dram_parameter("out", [B_CORE, 1], F32, isOutput=True)

    with tile.TileContext(nc) as tc:
        with (
            tc.tile_pool(name="consts", bufs=1) as consts,
            tc.tile_pool(name="bpool", bufs=3) as bpool,
            tc.tile_pool(name="xppool", bufs=2) as xppool,
            tc.tile_pool(name="z0pool", bufs=2) as z0pool,
            tc.tile_pool(name="ztpool", bufs=3) as ztpool,
            tc.tile_pool(name="zqpool", bufs=2) as zqpool,
            tc.tile_pool(name="hpool", bufs=2) as hpool,
            tc.tile_pool(name="ypool", bufs=4) as ypool,
            tc.tile_pool(name="spool", bufs=1) as spool,
            tc.tile_pool(name="ppool", bufs=6, space="PSUM") as ppool,
            tc.tile_pool(name="fcp", bufs=1, space="PSUM") as fcp,
        ):
            # --- small resident constants (first in the DMA queue) ---
            w0_sb = consts.tile([128, PAIR_CHUNKS, H], F16, tag="w0")
            nc.sync.dma_start(w0_sb[:], w0[:].rearrange("c k o -> k c o"))
            bia_sb = consts.tile([128, 3, 2], F32, tag="bia")
            nc.sync.dma_start(bia_sb[:], bia[:])
            fcw_sb = consts.tile([128, 4], F32, tag="fcw")
            nc.sync.dma_start(fcw_sb[:], fcw[:])
            fcb_sb = consts.tile([1, 1], F32, tag="fcb")
            nc.sync.dma_start(fcb_sb[:], fcb[:])

            def load_w(src, srcq, name):
                """Load one conv weight as 4 separately-DMA'd 8-m chunks
                (fp8 for chunks < NDR, fp16 above)."""
                chunks = []
                for j in range(4):
                    if j < NDR:
                        t = []
                        for oc in range(2):
                            tt = consts.tile(
                                [128, 8, H // 2], F8, tag=f"{name}q{j}_{oc}"
                            )
                            r0 = oc * NDR * 8 + 8 * j
                            nc.sync.dma_start(
                                tt[:],
                                srcq[r0 : r0 + 8].rearrange("c k o -> k c o"),
                            )
                            t.append(tt)
                    else:
                        t = consts.tile([128, 8, H], F16, tag=f"{name}_{j}")
                        nc.sync.dma_start(
                            t[:], src[8 * j : 8 * j + 8].rearrange("c k o -> k c o")
                        )
                    chunks.append(t)
                return chunks

            # per-item d-sums of the relu'd xs channels, [channel, item]
            s_tiles = [
                spool.tile([128, B_CORE], F32, tag=f"s{c}", name=f"s{c}")
                for c in range(4)
            ]

            h1s = [None] * N_GROUPS
            h2s = [None] * N_GROUPS
            Bgs = [None] * N_GROUPS
            w_chunks = [None, None]

            def stage_a(g):
                """layer 0 for group g: DMAs, z0 build, 10 matmuls, h1/y0."""
                i0 = g * GROUP
                # B[p, m, i, d] = x_{8g+i}[m, d] for every partition p,
                # 4 chunk DMAs of 8 m-values each (partition stride 0).
                Bgp = []
                for j in range(4):
                    Bj = bpool.tile([128, 8, GROUP, D], F16, tag=f"B{j}")
                    nc.sync.dma_start(
                        Bj[:],
                        bass.AP(
                            tensor=xg,
                            offset=g * M * GROUP * D + j * 8 * GROUP * D,
                            ap=[[0, 128], [1, 8 * GROUP * D]],
                        ),
                    )
                    Bgp.append(Bj)
                Bgs[g] = Bgp

                npair = PAIR_CHUNKS * GROUP * D
                xa = xppool.tile([128, PAIR_CHUNKS, GROUP, D], F16, tag="xa")
                nc.sync.dma_start(
                    xa[:],
                    bass.AP(
                        tensor=xpa,
                        offset=g * 128 * npair,
                        ap=[[npair, 128], [1, npair]],
                    ),
                )
                xb = xppool.tile([128, PAIR_CHUNKS, GROUP, D], F16, tag="xb")
                nc.sync.dma_start(
                    xb[:],
                    bass.AP(
                        tensor=xpb,
                        offset=g * 128 * npair,
                        ap=[[npair, 128], [1, npair]],
                    ),
                )

                # z0[p, c, i, d] = xa * xb  -- one full-width DVE op
                z0 = z0pool.tile([128, PAIR_CHUNKS, GROUP, D], F16, tag="z0")
                nc.vector.tensor_mul(z0[:], xa[:], xb[:])

                ps = [
                    ppool.tile([128, GROUP * D], F32, tag="yps", name=f"ps0_{g}_{oc}")
                    for oc in range(2)
                ]
                for c in range(PAIR_CHUNKS):
                    for oc in range(2):
                        nc.tensor.matmul(
                            ps[oc][:],
                            w0_sb[:, c, oc * 128 : (oc + 1) * 128],
                            z0[:, c, :, :],
                            start=(c == 0),
                            stop=(c == PAIR_CHUNKS - 1),
                        )

                # h-half -> h1 (feeds layer-1 z); x-half -> y0 + d-sums
                h1 = hpool.tile([128, GROUP, D], F16, tag="h1")
                nc.scalar.activation(h1[:], ps[1][:], RELU, bias=bia_sb[:, 0, 1:2])
                h1s[g] = h1
                y0 = ypool.tile([128, GROUP, D], F16, tag="y")
                nc.scalar.activation(y0[:], ps[0][:], RELU, bias=bia_sb[:, 0, 0:1])
                nc.vector.reduce_sum(s_tiles[0][:, i0 : i0 + GROUP], y0[:], axis=AXX)

            def stage_bc(g, lay):
                """layer 1 (lay=0) / layer 2 (lay=1) for group g."""
                i0 = g * GROUP
                wch = w_chunks[lay]
                h_in = h1s[g] if lay == 0 else h2s[g]
                Bgp = Bgs[g]
                ps = [
                    ppool.tile(
                        [128, GROUP * D], F32, tag="yps", name=f"ps{lay + 1}_{g}_{oc}"
                    )
                    for oc in range(2)
                ]
                for j in range(4):  # 4 sub-tiles of 8 k-chunks
                    zt = ztpool.tile([128, 8, GROUP, D], F16, tag="zt")
                    nc.vector.tensor_mul(
                        zt[:],
                        h_in[:, None, :, :].to_broadcast((128, 8, GROUP, D)),
                        Bgp[j][:],
                    )
                    if j < NDR:
                        # fp8 DoubleRow path: convert on ScalarE, 2 k-chunks
                        # per PE pass
                        zq = zqpool.tile([128, 8, GROUP, D], F8, tag="zq")
                        nc.scalar.activation(zq[:], zt[:], IDENT)
                        for mp in range(4):
                            m = 8 * j + 2 * mp
                            for oc in range(2):
                                nc.tensor.matmul(
                                    ps[oc][:],
                                    wch[j][oc][:, 2 * mp : 2 * mp + 2, :],
                                    zq[:, 2 * mp : 2 * mp + 2, :, :],
                                    start=(m == 0),
                                    stop=(m == 30),
                                    perf_mode=DR,
                                )
                    else:
                        for mm in range(8):
                            m = 8 * j + mm
                            for oc in range(2):
                                nc.tensor.matmul(
                                    ps[oc][:],
                                    wch[j][:, mm, oc * 128 : (oc + 1) * 128],
                                    zt[:, mm, :, :],
                                    start=(m == 0),
                                    stop=(m == 31),
                                )
                if lay == 0:
                    # split_half: x-half -> s1 sums, h-half -> h2
                    h2 = hpool.tile([128, GROUP, D], F16, tag="h2")
                    nc.scalar.activation(
                        h2[:], ps[1][:], RELU, bias=bia_sb[:, 1, 1:2], scale=1.0 / WS
                    )
                    h2s[g] = h2
                    y1 = ypool.tile([128, GROUP, D], F16, tag="y")
                    nc.scalar.activation(
                        y1[:], ps[0][:], RELU, bias=bia_sb[:, 1, 0:1], scale=1.0 / WS
                    )
                    nc.vector.reduce_sum(
                        s_tiles[1][:, i0 : i0 + GROUP], y1[:], axis=AXX
                    )
                else:
                    # last layer: both halves feed the FC sums (s2, s3)
                    for oc in range(2):
                        y2 = ypool.tile([128, GROUP, D], F16, tag="y")
                        nc.scalar.activation(
                            y2[:],
                            ps[oc][:],
                            RELU,
                            bias=bia_sb[:, 2, oc : oc + 1],
                            scale=1.0 / WS,
                        )
                        nc.vector.reduce_sum(
                            s_tiles[2 + oc][:, i0 : i0 + GROUP], y2[:], axis=AXX
                        )

            # 3-stage pipeline: A[t] | B[t-1] | C[t-2].  w1/w2 chunk DMAs are
            # queued behind the first/second group's input DMAs so the
            # earliest-needed bytes land first.
            for t in range(N_GROUPS + 2):
                if t < N_GROUPS:
                    stage_a(t)
                if t == 0:
                    w_chunks[0] = load_w(w1, w1q if NDR else None, "w1")
                if 1 <= t <= N_GROUPS:
                    stage_bc(t - 1, 0)
                if t == 1:
                    w_chunks[1] = load_w(w2, w2q if NDR else None, "w2")
                if t >= 2:
                    stage_bc(t - 2, 1)

            # ---------- final FC: out[i] = sum_c fcw[c] * s[c, i] + fcb ----------
            fc_ps = fcp.tile([1, B_CORE], F32, tag="fc")
            for c in range(4):
                nc.tensor.matmul(
                    fc_ps[:],
                    fcw_sb[:, c : c + 1],
                    s_tiles[c][:],
                    start=(c == 0),
                    stop=(c == 3),
                )
            osb = consts.tile([1, B_CORE], F32, tag="osb")
            nc.scalar.activation(osb[:], fc_ps[:], IDENT, bias=fcb_sb[0:1, 0:1])
            nc.sync.dma_start(out[:], osb[:])

    _legalize_waits(nc)
    return nc


def _legalize_waits(nc, max_waits=1):
    """walrus codegen allows at most 2 semaphore waits per instruction; spill
    the excess onto NoOps injected just before the offender on the same
    engine (same-engine FIFO makes this ordering-equivalent)."""
    for bb in nc.main_func.blocks:
        insts = bb.instructions
        new_list = []
        changed = False
        for ins in insts:
            si = ins.sync_info
            if si is not None and si.on_wait and len(si.on_wait) > max_waits:
                waits = list(si.on_wait)
                extra, keep = waits[:-max_waits], waits[-max_waits:]
                k = 0
                while k < len(extra):
                    chunk = extra[k : k + max_waits]
                    nop = mybir.InstNoOp(name=f"{ins.name}-w{k}", ins=[], outs=[])
                    nop.engine = ins.engine
                    nop.sync_info = mybir.SyncInfo(on_wait=chunk, on_update=[])
                    new_list.append(nop)
                    k += max_waits
                ins.sync_info = mybir.SyncInfo(
                    on_wait=keep,
                    on_update=list(si.on_update) if si.on_update else [],
                )
                changed = True
            new_list.append(ins)
        if changed:
            if hasattr(bb, "set_instructions"):
                bb.set_instructions(new_list)
            else:
                insts.clear()
                insts.extend(new_list)
                if len(bb.instructions) != len(new_list):
                    bb.instructions = new_list


def _sym_pairs():
    """Enumeration of the 528 unique (a<=b) field pairs, zero-padded to 640."""
    ia, ib = [], []
    for a in range(M):
        for b in range(a, M):
            ia.append(a)
            ib.append(b)
    pad = PAIR_CHUNKS * 128 - len(ia)
    ia += [0] * pad
    ib += [0] * pad
    return np.asarray(ia), np.asarray(ib), pad


def _to_fp8(w):
    return np.clip(w, -240.0, 240.0).astype(ml_dtypes.float8_e4m3)


def prep_inputs(x, W0, b0, W1, b1, W2, b2, fc_W, fc_b):
    """Host-side reshape/cast into the per-core input maps."""
    xh = np.ascontiguousarray(np.asarray(x).astype(np.float16))
    ia, ib, pad = _sym_pairs()
    ng_all = B_TOTAL // GROUP
    # xg[g, m, i, d] = x[8g + i, m, d]
    xg = np.ascontiguousarray(
        xh.reshape(ng_all, GROUP, M, D).transpose(0, 2, 1, 3)
    ).reshape(ng_all, M * GROUP * D)
    # xpa[g, p, c, i, d] = x[8g + i, ia[128c + p], d]
    xpa = np.ascontiguousarray(
        xh[:, ia, :]
        .reshape(ng_all, GROUP, PAIR_CHUNKS, 128, D)
        .transpose(0, 3, 2, 1, 4)
    ).reshape(ng_all, 128, PAIR_CHUNKS * GROUP * D)
    xpb = np.ascontiguousarray(
        xh[:, ib, :]
        .reshape(ng_all, GROUP, PAIR_CHUNKS, 128, D)
        .transpose(0, 3, 2, 1, 4)
    ).reshape(ng_all, 128, PAIR_CHUNKS * GROUP * D)
    # fold W0 over the (a,b)<->(b,a) symmetry: row (a,b) gets W0[a*32+b]
    # (+ W0[b*32+a] when a != b); padded rows are zero.
    W0 = np.asarray(W0, dtype=np.float64)
    w0f = W0[ia * M + ib] + np.where((ia != ib)[:, None], W0[ib * M + ia], 0.0)
    w0f[len(ia) - pad :] = 0.0
    w0f = np.ascontiguousarray(
        w0f.astype(np.float16).reshape(PAIR_CHUNKS, 128, H)
    )
    W1s = np.asarray(W1, np.float64) * WS
    W2s = np.asarray(W2, np.float64) * WS
    w1 = np.ascontiguousarray(W1s.astype(np.float16).reshape(32, 128, H))
    w2 = np.ascontiguousarray(W2s.astype(np.float16).reshape(32, 128, H))
    bia = np.ascontiguousarray(
        np.stack([b0, b1, b2]).reshape(3, 2, 128).transpose(2, 0, 1).astype(np.float32)
    )
    fcw = np.ascontiguousarray(np.asarray(fc_W).reshape(4, 128).T.astype(np.float32))
    fcb = np.ascontiguousarray(np.asarray(fc_b).reshape(1, 1).astype(np.float32))
    shared = {"w0": w0f, "w1": w1, "w2": w2, "bia": bia, "fcw": fcw, "fcb": fcb}
    if NDR:
        # oc-major split: [oc*(NDR*8) + c, k, 128]
        shared["w1q"] = np.ascontiguousarray(
            _to_fp8(
                np.concatenate(
                    [w1[: NDR * 8, :, :128], w1[: NDR * 8, :, 128:]], axis=0
                ).astype(np.float32)
            )
        )
        shared["w2q"] = np.ascontiguousarray(
            _to_fp8(
                np.concatenate(
                    [w2[: NDR * 8, :, :128], w2[: NDR * 8, :, 128:]], axis=0
                ).astype(np.float32)
            )
        )
    return [
        {
            "xg": xg[i * N_GROUPS : (i + 1) * N_GROUPS],
            "xpa": xpa[i * N_GROUPS : (i + 1) * N_GROUPS],
            "xpb": xpb[i * N_GROUPS : (i + 1) * N_GROUPS],
            **shared,
        }
        for i in range(N_CORES)
    ]


_NC = None


def _get_nc():
    global _NC
    if _NC is None:
        _NC = build()
    return _NC


def kernel(**inputs):
    in_maps = prep_inputs(**inputs)
    res = run_bass_kernel_spmd(_get_nc(), in_maps, list(range(N_CORES)))
    return np.ascontiguousarray(
        np.concatenate([r["out"] for r in res.results], axis=0).astype(np.float32)
    )
